# revision 25
# baseline (speedup 1.0000x reference)
"""Trainium2 Bass kernel for nn_AssociativeMemoryStep (forward-looking retention).

reference semantics:
    q,k,v,o weights = basis @ {q,k,v,o}_coeffs.T          [V, C]
    q/k/v = x @ w                                         [B, T, C]
    scores[t,s] = (q_t . k_s) * decay^(s-t-1) for s>t     (anti-causal)
    retrieved = scores @ v ; out = retrieved @ o_w.T * out_scale

All four projections factor through the shared 2*n_basis=256-dim basis:
    z = x @ basis [T, M];  q = z @ qc^T, k = z @ kc^T, v = z @ vc^T
so the V=1024 contraction is done ONCE (zT) and the per-head projections are
cheap M=256 contractions. Further, q and k only ever appear as inner products
(scores and state), so they fold into one Gram projection via the
host-precomputed G_a = kc^T @ qc [M, M]:
    k_j . q_i = z_j G_a z_i^T  ->  g = z @ G_a;  A = g z^T
    cross = q~ S = z~ S^~ with S^~ = g~^T v  (g~ = decay^t * g, z~ = crossb * z)
eliminating the separate q and k projections entirely.

Chunkwise-recurrent backward retention with state S = sum_{s in next chunk}
decay^(s_local) k_s^T v_s (size [C,C]), chunk L=256; per-chunk state
truncation (decay^256 ~ 4e-6) makes chunks independent. Intra-chunk masked
attention is triangle-trimmed: the j<128,i>=128 quarter of each 256x256 score
tile is identically zero and never computed.

Sharding: 8 cores = 4 batches x 2 sequence halves of T_loc=2048. Each core
gets a HALO=128 slice of the next half's x and recomputes the boundary state
locally (truncation ~decay^128 ~ 2e-3, well under the 2e-2 gate).

Device layout fully transposed: zT/qT/kT/rT are [dim, T]; k~ (decay-scaled)
and v are computed together in one fused [t,512] stream per 128-pos block
(no PE transposes). Output produced as outT [V, T_loc], transposed on host.
All DRAM inputs host-pre-tiled into SBUF-contiguous 2D blocks.
"""

import numpy as np
import ml_dtypes

import concourse.bass as bass
import concourse.mybir as mybir
import concourse.tile as tile
from concourse import bacc
from concourse.bass_utils import run_bass_kernel_spmd

BF16 = ml_dtypes.bfloat16

B, T, V, C = 4, 4096, 1024, 256
M = 256               # 2 * n_basis
N_CORES = 8
T_LOC = 2048          # main positions per core
HALO = 128            # halo positions (state-only; decay^128 ~ 2e-3 truncation)
T_EXT = T_LOC + HALO
L = 256               # retention chunk
PCH = 512             # projection t-chunk
N_MAIN_PCH = T_LOC // PCH  # 4
N_MAIN_CH = T_LOC // L     # 8
KT = V // 128         # 8 v-ktiles
CT = C // 128         # 2 c-tiles
MT = M // 128         # 2 m-tiles
NTT = T_EXT // 128    # 17 t-tiles (16 main + 1 halo)

FP32 = mybir.dt.float32
BF = mybir.dt.bfloat16


def build_nc():
    nc = bacc.Bacc("TRN2", target_bir_lowering=False, debug=False,
                   num_devices=N_CORES)

    # all inputs host-pre-tiled to be contiguous per [128, N] DMA block
    xh_d = nc.dram_tensor("xh", [N_MAIN_PCH * 128, KT * PCH], BF, kind="ExternalInput")
    xhh_d = nc.dram_tensor("xhh", [128, KT * HALO], BF, kind="ExternalInput")
    bss_d = nc.dram_tensor("bss", [128, KT * M], BF, kind="ExternalInput")
    # coef blocks: [mt][j][c] with j in (G_a, vc^T); adjacent for fused g|v
    coef_d = nc.dram_tensor("coef", [128, MT * 2 * C], BF, kind="ExternalInput")
    owT_d = nc.dram_tensor("owT", [128, CT * V], BF, kind="ExternalInput")
    maskT_d = nc.dram_tensor("maskT", [128, CT * L], FP32, kind="ExternalInput")
    crossb_d = nc.dram_tensor("crossb", [128, PCH], FP32, kind="ExternalInput")
    kscale_d = nc.dram_tensor("kscale", [128, 2], FP32, kind="ExternalInput")
    outT_d = nc.dram_tensor("outT", [V, T_LOC], BF, kind="ExternalOutput")

    with tile.TileContext(nc) as tc:
        build_tile(tc, xh_d, xhh_d, bss_d, coef_d, owT_d, maskT_d, crossb_d,
                   kscale_d, outT_d)
    nc.compile()
    return nc


def build_tile(tc, xh_d, xhh_d, bss_d, coef_d, owT_d, maskT_d, crossb_d,
               kscale_d, outT_d):
    nc = tc.nc

    import contextlib
    ctx = contextlib.ExitStack()
    consts = ctx.enter_context(tc.tile_pool(name="consts", bufs=1))
    xpool = ctx.enter_context(tc.tile_pool(name="xpool", bufs=3))
    big = ctx.enter_context(tc.tile_pool(name="big", bufs=1))
    atmp = ctx.enter_context(tc.tile_pool(name="atmp", bufs=4))
    state = ctx.enter_context(tc.tile_pool(name="state", bufs=3))
    ostage = ctx.enter_context(tc.tile_pool(name="ostage", bufs=6))
    psA = ctx.enter_context(tc.tile_pool(name="psA", bufs=3, space="PSUM"))
    psB = ctx.enter_context(tc.tile_pool(name="psB", bufs=2, space="PSUM"))
    psO = ctx.enter_context(tc.tile_pool(name="psO", bufs=3, space="PSUM"))

    # ---- constant tiles; DMA order = need order ----
    # basis split in four tiles (2 kt each) so the first zT matmuls gate on
    # 128KB (Tile deps are tile-granular)
    bss_sb = [consts.tile([128, 2, M], BF, name=f"bss{i}") for i in range(4)]
    coef_sb = consts.tile([128, MT, 2, C], BF)
    owT_sb = consts.tile([128, CT * V], BF)
    maskT_sb = consts.tile([128, CT, L], FP32)
    crossb_sb = consts.tile([128, PCH], FP32)
    kscale_sb = consts.tile([128, 2], FP32)

    def dma_split(out_tile, in_ap, n, engines=(None,)):
        # split one big contiguous DMA into n pieces so they spread across
        # HWDGE queues (aggregate bandwidth), alternating the issuing engine
        # (each dma_start costs ~650ns serialized on its sequencer)
        if engines == (None,):
            engines = (nc.sync, nc.gpsimd)
        if len(out_tile.shape) == 3:
            g = out_tile.shape[1] // n
            b = out_tile.shape[2]
            for i in range(n):
                engines[i % len(engines)].dma_start(
                    out=out_tile[:, i * g:(i + 1) * g, :],
                    in_=in_ap[:, i * g * b:(i + 1) * g * b].rearrange(
                        "p (a b) -> p a b", b=b))
            return
        w = out_tile.shape[-1] // n
        for i in range(n):
            engines[i % len(engines)].dma_start(
                out=out_tile[:, i * w:(i + 1) * w],
                in_=in_ap[:, i * w:(i + 1) * w])

    # ---- persistent activations ----
    zT_sb = big.tile([128, MT, T_EXT], BF)     # [m, t] shared projection
    gT_sb = big.tile([128, MT, T_LOC], BF)     # [m, t] g = z @ G_a, main only
    z1T_sb = big.tile([128, MT, T_LOC], BF)    # crossb-scaled z~T
    gtil_sb = big.tile([128, NTT, M], BF)      # g~ = g * decay^t_local, [t, m]
    v_sb = big.tile([128, NTT, C], BF)         # v normal layout
    rT_sb = big.tile([128, CT, T_LOC], BF)     # retrieved^T

    # ---- startup DMAs: halo-x + first basis tiles first (gate the first
    # matmuls on the fewest bytes) ----
    start_engines = [nc.sync, nc.gpsimd, nc.scalar]
    xth = xpool.tile([128, KT * HALO], BF, tag="xth")
    pieces = [(xth[:, :KT * HALO // 2], xhh_d.ap()[:, :KT * HALO // 2]),
              (xth[:, KT * HALO // 2:], xhh_d.ap()[:, KT * HALO // 2:])]
    for i in range(4):
        pieces.append((bss_sb[i],
                       bss_d.ap()[:, i * 2 * M:(i + 1) * 2 * M].rearrange(
                           "p (a b) -> p a b", b=M)))
    for i in range(2):
        g = coef_d.ap().shape[1] // 2
        pieces.append((coef_sb[:, i, :, :],
                       coef_d.ap()[:, i * g:(i + 1) * g].rearrange(
                           "p (a b) -> p a b", b=C)))
    pieces.append((kscale_sb, kscale_d.ap()))
    for i, (dst, src) in enumerate(pieces):
        start_engines[i % 3].dma_start(out=dst, in_=src)

    # ---- phase 0: halo (128 pos): zT then fused k~|v then boundary state ----
    tt_h = T_LOC // 128  # halo t-tile index (16)
    for mt in range(MT):
        ps = psA.tile([128, HALO], FP32, tag="ps")
        for kt in range(KT):
            nc.tensor.matmul(
                ps, lhsT=bss_sb[kt // 2][:, kt % 2, mt * 128:(mt + 1) * 128],
                rhs=xth[:, kt * HALO:(kt + 1) * HALO],
                start=(kt == 0), stop=(kt == KT - 1))
        (nc.vector.tensor_copy if mt == 0 else nc.scalar.copy)(
            zT_sb[:, mt, T_LOC:T_EXT], ps)
    # fused g~|v for the halo block
    ps = psA.tile([128, 2 * C], FP32, tag="ps")
    for mt in range(MT):
        nc.tensor.matmul(
            ps, lhsT=zT_sb[:, mt, T_LOC:T_EXT],
            rhs=coef_sb[:, mt, :, :].rearrange("p a b -> p (a b)"),
            start=(mt == 0), stop=(mt == MT - 1))
    nc.vector.tensor_scalar_mul(
        gtil_sb[:, tt_h, :], ps[:, :M], kscale_sb[:, 0:1])
    nc.scalar.copy(v_sb[:, tt_h, :], ps[:, M:])
    # boundary state S~ = g~^T v over the halo block only
    S_cur = state.tile([128, MT, C], BF, tag="S")
    for st in range(MT):
        ps = psA.tile([128, C], FP32, tag="ps")
        nc.tensor.matmul(
            ps, lhsT=gtil_sb[:, tt_h, st * 128:(st + 1) * 128],
            rhs=v_sb[:, tt_h, :], start=True, stop=True)
        (nc.vector.tensor_copy if st == 0 else nc.scalar.copy)(
            S_cur[:, st, :], ps)

    # ---- main loop: projections (reverse t), retention, outproj ----
    def proj(pch, xt_a, xt_b):
        t0 = pch * PCH
        # zT: [m, t] via lhsT=basis tiles, rhs=xT
        # zT evacuated as two half copies on two engines so the dependent
        # gT/g~|v matmuls see bf16 zT ~350ns after the psum closes; z~T is
        # re-derived from SBUF off the critical path
        for mt in range(MT):
            ps = psA.tile([128, PCH], FP32, tag="ps")
            for kt in range(KT):
                xt = xt_a if kt < 4 else xt_b
                nc.tensor.matmul(
                    ps, lhsT=bss_sb[kt // 2][:, kt % 2, mt * 128:(mt + 1) * 128],
                    rhs=xt[:, (kt % 4) * PCH:(kt % 4 + 1) * PCH],
                    start=(kt == 0), stop=(kt == KT - 1))
            h = PCH // 2
            nc.scalar.copy(zT_sb[:, mt, t0:t0 + h], ps[:, :h])
            nc.vector.tensor_copy(zT_sb[:, mt, t0 + h:t0 + PCH], ps[:, h:])
            nc.vector.tensor_mul(
                z1T_sb[:, mt, t0:t0 + PCH], zT_sb[:, mt, t0:t0 + PCH],
                crossb_sb)
        # gT: [m, t] via lhsT=G_a blocks, rhs=zT; mt-major so the mt=1
        # accumulations land only after zT[mt=1]'s evacuation is done anyway
        gps = [psA.tile([128, PCH], FP32, tag="ps", name=f"gps{i}")
               for i in range(MT)]
        for mt in range(MT):
            for mmt in range(MT):
                nc.tensor.matmul(
                    gps[mmt], lhsT=coef_sb[:, mt, 0, mmt * 128:(mmt + 1) * 128],
                    rhs=zT_sb[:, mt, t0:t0 + PCH],
                    start=(mt == 0), stop=(mt == MT - 1))
        for mmt in range(MT):
            (nc.scalar.copy if mmt == 0 else nc.vector.tensor_copy)(
                gT_sb[:, mmt, t0:t0 + PCH], gps[mmt])
        # fused g~|v in normal [t, m] layout: lhsT=zT t-blocks, rhs=[G_a|vc],
        # mt-major in tb pairs through the (projection-idle) psB slots
        for tp in range(2):
            tbs = (2 * tp, 2 * tp + 1)
            kps = [psB.tile([128, 2 * C], FP32, tag="ps", name=f"kps{tb}")
                   for tb in tbs]
            for mt in range(MT):
                for i, tb in enumerate(tbs):
                    nc.tensor.matmul(
                        kps[i],
                        lhsT=zT_sb[:, mt, t0 + tb * 128:t0 + (tb + 1) * 128],
                        rhs=coef_sb[:, mt, :, :].rearrange("p a b -> p (a b)"),
                        start=(mt == 0), stop=(mt == MT - 1))
            for i, tb in enumerate(tbs):
                tt = t0 // 128 + tb
                nc.vector.tensor_scalar_mul(
                    gtil_sb[:, tt, :], kps[i][:, :M],
                    kscale_sb[:, (tt % 2):(tt % 2) + 1])
                nc.scalar.copy(v_sb[:, tt, :], kps[i][:, M:])

    def retention_chunk(c):
        nonlocal S_cur
        c0 = c * L
        tt0 = c0 // 128  # first t-tile of this chunk (2 per chunk)
        # AT[j, i] = g_j . z_i ; masked -> atm (bf16).
        # jt=0 only needs i<128 (the j<128,i>=128 quarter is masked to zero).
        atm = []
        for jt in range(2):
            w = 128 if jt == 0 else L
            ps = psA.tile([128, w], FP32, tag="ps")
            for mt in range(MT):
                nc.tensor.matmul(
                    ps, lhsT=gT_sb[:, mt, c0 + jt * 128:c0 + (jt + 1) * 128],
                    rhs=zT_sb[:, mt, c0:c0 + w],
                    start=(mt == 0), stop=(mt == MT - 1))
            am = atmp.tile([128, w], BF, tag="atm")
            nc.vector.tensor_mul(am, ps, maskT_sb[:, jt, :w])
            atm.append(am)
        # cross part first: S~^T @ z~T needs no masks, so PE keeps running
        # while DVE applies them
        rps = []
        for ct in range(CT):
            ps = psB.tile([128, L], FP32, tag="ps")
            for st in range(MT):
                nc.tensor.matmul(
                    ps, lhsT=S_cur[:, st, ct * 128:(ct + 1) * 128],
                    rhs=z1T_sb[:, st, c0:c0 + L],
                    start=(st == 0), stop=False)
            rps.append(ps)
        # state S~ = g~^T v of THIS chunk only (decay^L * older state ~ 4e-6,
        # numerically negligible -> no recursion, chunks fully independent).
        # Emitted between the cross and intra parts as mask-independent filler.
        if c > 0:
            S_new = state.tile([128, MT, C], BF, tag="S")
            for st in range(MT):
                ps = psA.tile([128, C], FP32, tag="ps")
                for jt in range(2):
                    nc.tensor.matmul(
                        ps, lhsT=gtil_sb[:, tt0 + jt, st * 128:(st + 1) * 128],
                        rhs=v_sb[:, tt0 + jt, :],
                        start=(jt == 0), stop=(jt == 1))
                (nc.vector.tensor_copy if st == 0 else nc.scalar.copy)(
                    S_new[:, st, :], ps)
            S_cur = S_new
        # intra part: v^T @ atm accumulated into the same rT psum; jt=0 only
        # contributes to the first 128 columns
        for ct in range(CT):
            nc.tensor.matmul(
                rps[ct][:, :128], lhsT=v_sb[:, tt0, ct * 128:(ct + 1) * 128],
                rhs=atm[0], start=False, stop=False)
            nc.tensor.matmul(
                rps[ct], lhsT=v_sb[:, tt0 + 1, ct * 128:(ct + 1) * 128],
                rhs=atm[1], start=False, stop=True)
            (nc.vector.tensor_copy if ct == 0 else nc.scalar.copy)(
                rT_sb[:, ct, c0:c0 + L], rps[ct])

    def outproj(t0, width, final=False):
        # out[v, t0:t0+width]: owT^T @ rT; vt-pairs share one DMA (a 3D
        # [128, 2, w] staging tile maps onto 256 contiguous DRAM rows)
        for vt in range(KT):
            ps = psO.tile([128, PCH], FP32, tag="po")
            for ct in range(CT):
                nc.tensor.matmul(
                    ps[:, :width],
                    lhsT=owT_sb[:, ct * V + vt * 128:ct * V + (vt + 1) * 128],
                    rhs=rT_sb[:, ct, t0:t0 + width],
                    start=(ct == 0), stop=(ct == CT - 1))
            if vt % 2 == 0:
                ot = ostage.tile([128, 2, PCH], BF, tag="ot")
            # alternate evac engines so consecutive psO slots drain on two
            # engines concurrently
            (nc.vector.tensor_copy if vt % 2 == 0 else nc.scalar.copy)(
                ot[:, vt % 2, :width], ps[:, :width])
            if vt % 2 == 1:
                # the tail stays on sync HWDGE: a trailing gpsimd DMA makes
                # the kernel-end gpsimd DRAIN wait ~4us for SWDGE completion
                eng = nc.sync if (final or vt % 4 == 1) else nc.gpsimd
                eng.dma_start(
                    out=outT_d.ap()[(vt - 1) * 128:(vt + 1) * 128,
                                    t0:t0 + width].rearrange(
                        "(a p) b -> p a b", p=128),
                    in_=ot[:, :, :width])

    # x for pch=3 in two tiles so zT kt 0-3 gates on half the bytes
    xt3a = xpool.tile([128, 4 * PCH], BF, tag="xta")
    xt3b = xpool.tile([128, 4 * PCH], BF, tag="xtb")
    dma_split(xt3a, xh_d.ap()[3 * 128:4 * 128, :4 * PCH], 4,
              engines=(nc.sync, nc.gpsimd, nc.scalar))
    dma_split(xt3b, xh_d.ap()[3 * 128:4 * 128, 4 * PCH:], 4,
              engines=(nc.gpsimd, nc.sync))
    nc.gpsimd.dma_start(out=crossb_sb, in_=crossb_d.ap())
    nc.scalar.dma_start(out=maskT_sb,
                        in_=maskT_d.ap().rearrange("p (a b) -> p a b", b=L))

    # prefetch ALL remaining x chunks now (xpool holds 3 "xt" tiles), with
    # owT (first needed ~35us in) after xt1 in queue order
    xts = {3: (xt3a, xt3b)}
    for pch in (2, 1):
        xt = xpool.tile([128, KT * PCH], BF, tag="xt")
        dma_split(xt, xh_d.ap()[pch * 128:(pch + 1) * 128, :], 8)
        xts[pch] = (xt[:, :4 * PCH], xt[:, 4 * PCH:])
    dma_split(owT_sb, owT_d.ap(), 4)
    xt = xpool.tile([128, KT * PCH], BF, tag="xt")
    dma_split(xt, xh_d.ap()[0:128, :], 8)
    xts[0] = (xt[:, :4 * PCH], xt[:, 4 * PCH:])
    for pch in range(N_MAIN_PCH - 1, -1, -1):
        proj(pch, *xts[pch])
        if pch == N_MAIN_PCH - 1:
            retention_chunk(7)
            retention_chunk(6)
        else:
            # outproj for the previous pch's chunks, emitted one chunk after
            # its rT inputs complete so the rT evacuations have slack
            retention_chunk(2 * pch + 1)
            outproj((pch + 1) * PCH, PCH)
            retention_chunk(2 * pch)
    # tail at L granularity so the post-retention drain is short
    outproj(L, L)
    outproj(0, L, final=True)

    ctx.close()


# ---------------- host side ----------------

_NC_CACHE = None


def _get_nc():
    global _NC_CACHE
    if _NC_CACHE is None:
        _NC_CACHE = build_nc()
    return _NC_CACHE


def _tile128(a, inner):
    """[G*128, inner] -> [128, G*inner] with block g at columns [g*inner,...)."""
    g = a.shape[0] // 128
    return np.ascontiguousarray(
        a.reshape(g, 128, inner).transpose(1, 0, 2).reshape(128, g * inner))


def _prep_in_maps(inputs):
    x = np.asarray(inputs["x"], np.float32)
    basis = np.asarray(inputs["basis"], np.float32)
    decay = float(1.0 / (1.0 + np.exp(-np.float64(inputs["decay_logit"]))))
    out_scale = float(np.float32(inputs["out_scale"]))

    bss = _tile128(basis.astype(BF16), M)
    # coef[p, mt, 0, :] = G_a[mt*128+p, :] with G_a = kc^T @ qc;
    # coef[p, mt, 1, :] = vc^T[mt*128+p, :]
    qc = np.asarray(inputs["q_coeffs"], np.float32)
    kc = np.asarray(inputs["k_coeffs"], np.float32)
    vc = np.asarray(inputs["v_coeffs"], np.float32)
    Ga = (kc.T @ qc).astype(BF16)              # [M, M]
    coef = np.empty((128, MT, 2, C), dtype=BF16)
    for mt in range(MT):
        coef[:, mt, 0, :] = Ga[mt * 128:(mt + 1) * 128, :]
        coef[:, mt, 1, :] = vc.T[mt * 128:(mt + 1) * 128, :].astype(BF16)
    coef = np.ascontiguousarray(coef).reshape(128, MT * 2 * C)
    ow = basis @ np.asarray(inputs["o_coeffs"], np.float32).T
    owT = _tile128(np.ascontiguousarray((ow * out_scale).T).astype(BF16), V)

    i = np.arange(L)
    jj, ii = np.meshgrid(i, i, indexing="ij")
    maskT = np.where(jj > ii, decay ** np.maximum(jj - ii - 1, 0), 0.0).astype(np.float32)
    maskT = _tile128(maskT, L)
    cross = (decay ** (L - 1 - i)).astype(np.float32)
    crossb = np.ascontiguousarray(
        np.broadcast_to(np.tile(cross, PCH // L)[None, :], (128, PCH)), np.float32)
    ksc = decay ** np.arange(2 * 128, dtype=np.float64)
    kscale = np.stack([ksc[:128], ksc[128:]], axis=1).astype(np.float32)

    in_maps = []
    for core in range(N_CORES):
        b, h = divmod(core, 2)
        t0 = h * T_LOC
        te = min(t0 + T_EXT, T)
        xT = np.zeros((V, T_EXT), dtype=BF16)
        xT[:, :te - t0] = x[b, t0:te].T.astype(BF16)
        # pre-tile main: [pch][p][kt][t] contiguous; halo: [p][kt][t]
        xh = np.ascontiguousarray(
            xT[:, :T_LOC].reshape(KT, 128, N_MAIN_PCH, PCH).transpose(2, 1, 0, 3)
        ).reshape(N_MAIN_PCH * 128, KT * PCH)
        xhh = np.ascontiguousarray(
            xT[:, T_LOC:].reshape(KT, 128, HALO).transpose(1, 0, 2)
        ).reshape(128, KT * HALO)
        in_maps.append({
            "xh": xh, "xhh": xhh, "bss": bss, "coef": coef, "owT": owT,
            "maskT": maskT, "crossb": crossb, "kscale": kscale,
        })
    return in_maps


def _ensure_ntff_hook():
    """The agent image's antenv package lacks axon_hooks; shim it so
    run_bass_kernel_spmd(trace=True) can register the NTFF profile hook."""
    try:
        from antenv.axon_hooks import get_axon_ntff_profile_hook  # noqa: F401
        return
    except ImportError:
        pass
    import sys
    import types
    import antenv
    mod = types.ModuleType("antenv.axon_hooks")
    _state = {"hook": None}
    mod.set_axon_ntff_profile_hook = lambda h: _state.__setitem__("hook", h)
    mod.get_axon_ntff_profile_hook = lambda: _state["hook"]
    sys.modules["antenv.axon_hooks"] = mod
    antenv.axon_hooks = mod
    from trn_agent_boot.trn_boot import _ntff_profile_via_ctypes
    mod.set_axon_ntff_profile_hook(
        _ntff_profile_via_ctypes("/opt/axon/libaxon_pjrt.so"))


def run(inputs, trace=False):
    """Returns (out [B,T,V] float32, BassKernelResults)."""
    if trace:
        _ensure_ntff_hook()
    in_maps = _prep_in_maps(inputs)
    nc = _get_nc()
    res = run_bass_kernel_spmd(nc, in_maps, core_ids=list(range(N_CORES)),
                               trace=trace)
    out = np.zeros((B, T, V), np.float32)
    for core in range(N_CORES):
        b, h = divmod(core, 2)
        outT = np.asarray(res.results[core]["outT"]).astype(np.float32)
        out[b, h * T_LOC:(h + 1) * T_LOC] = outT.T
    return out, res


def kernel(**inputs):
    out, _ = run(inputs, trace=False)
    return out


# revision 26
# speedup vs baseline: 1.0021x; 1.0021x over previous
"""Trainium2 Bass kernel for nn_AssociativeMemoryStep (forward-looking retention).

reference semantics:
    q,k,v,o weights = basis @ {q,k,v,o}_coeffs.T          [V, C]
    q/k/v = x @ w                                         [B, T, C]
    scores[t,s] = (q_t . k_s) * decay^(s-t-1) for s>t     (anti-causal)
    retrieved = scores @ v ; out = retrieved @ o_w.T * out_scale

All four projections factor through the shared 2*n_basis=256-dim basis:
    z = x @ basis [T, M];  q = z @ qc^T, k = z @ kc^T, v = z @ vc^T
so the V=1024 contraction is done ONCE (zT) and the per-head projections are
cheap M=256 contractions. Further, q and k only ever appear as inner products
(scores and state), so they fold into one Gram projection via the
host-precomputed G_a = kc^T @ qc [M, M]:
    k_j . q_i = z_j G_a z_i^T  ->  g = z @ G_a;  A = g z^T
    cross = q~ S = z~ S^~ with S^~ = g~^T v  (g~ = decay^t * g, z~ = crossb * z)
eliminating the separate q and k projections entirely.

Chunkwise-recurrent backward retention with state S = sum_{s in next chunk}
decay^(s_local) k_s^T v_s (size [C,C]), chunk L=256; per-chunk state
truncation (decay^256 ~ 4e-6) makes chunks independent. Intra-chunk masked
attention is triangle-trimmed: the j<128,i>=128 quarter of each 256x256 score
tile is identically zero and never computed.

Sharding: 8 cores = 4 batches x 2 sequence halves of T_loc=2048. Each core
gets a HALO=128 slice of the next half's x and recomputes the boundary state
locally (truncation ~decay^128 ~ 2e-3, well under the 2e-2 gate).

Device layout fully transposed: zT/qT/kT/rT are [dim, T]; k~ (decay-scaled)
and v are computed together in one fused [t,512] stream per 128-pos block
(no PE transposes). Output produced as outT [V, T_loc], transposed on host.
All DRAM inputs host-pre-tiled into SBUF-contiguous 2D blocks.
"""

import numpy as np
import ml_dtypes

import concourse.bass as bass
import concourse.mybir as mybir
import concourse.tile as tile
from concourse import bacc
from concourse.bass_utils import run_bass_kernel_spmd

BF16 = ml_dtypes.bfloat16

B, T, V, C = 4, 4096, 1024, 256
M = 256               # 2 * n_basis
N_CORES = 8
T_LOC = 2048          # main positions per core
HALO = 128            # halo positions (state-only; decay^128 ~ 2e-3 truncation)
T_EXT = T_LOC + HALO
L = 256               # retention chunk
PCH = 512             # projection t-chunk
N_MAIN_PCH = T_LOC // PCH  # 4
N_MAIN_CH = T_LOC // L     # 8
KT = V // 128         # 8 v-ktiles
CT = C // 128         # 2 c-tiles
MT = M // 128         # 2 m-tiles
NTT = T_EXT // 128    # 17 t-tiles (16 main + 1 halo)

FP32 = mybir.dt.float32
BF = mybir.dt.bfloat16


def build_nc():
    nc = bacc.Bacc("TRN2", target_bir_lowering=False, debug=False,
                   num_devices=N_CORES)

    # all inputs host-pre-tiled to be contiguous per [128, N] DMA block
    xh_d = nc.dram_tensor("xh", [N_MAIN_PCH * 128, KT * PCH], BF, kind="ExternalInput")
    xhh_d = nc.dram_tensor("xhh", [128, KT * HALO], BF, kind="ExternalInput")
    bss_d = nc.dram_tensor("bss", [128, KT * M], BF, kind="ExternalInput")
    # coef blocks: [mt][j][c] with j in (G_a, vc^T); adjacent for fused g|v
    coef_d = nc.dram_tensor("coef", [128, MT * 2 * C], BF, kind="ExternalInput")
    owT_d = nc.dram_tensor("owT", [128, CT * V], BF, kind="ExternalInput")
    maskT_d = nc.dram_tensor("maskT", [128, CT * L], FP32, kind="ExternalInput")
    crossb_d = nc.dram_tensor("crossb", [128, PCH], FP32, kind="ExternalInput")
    kscale_d = nc.dram_tensor("kscale", [128, 2], FP32, kind="ExternalInput")
    outT_d = nc.dram_tensor("outT", [V, T_LOC], BF, kind="ExternalOutput")

    with tile.TileContext(nc) as tc:
        build_tile(tc, xh_d, xhh_d, bss_d, coef_d, owT_d, maskT_d, crossb_d,
                   kscale_d, outT_d)
    nc.compile()
    return nc


def build_tile(tc, xh_d, xhh_d, bss_d, coef_d, owT_d, maskT_d, crossb_d,
               kscale_d, outT_d):
    nc = tc.nc

    import contextlib
    ctx = contextlib.ExitStack()
    consts = ctx.enter_context(tc.tile_pool(name="consts", bufs=1))
    xpool = ctx.enter_context(tc.tile_pool(name="xpool", bufs=3))
    big = ctx.enter_context(tc.tile_pool(name="big", bufs=1))
    atmp = ctx.enter_context(tc.tile_pool(name="atmp", bufs=4))
    state = ctx.enter_context(tc.tile_pool(name="state", bufs=3))
    ostage = ctx.enter_context(tc.tile_pool(name="ostage", bufs=6))
    psA = ctx.enter_context(tc.tile_pool(name="psA", bufs=3, space="PSUM"))
    psB = ctx.enter_context(tc.tile_pool(name="psB", bufs=2, space="PSUM"))
    psO = ctx.enter_context(tc.tile_pool(name="psO", bufs=3, space="PSUM"))

    # ---- constant tiles; DMA order = need order ----
    # basis split in four tiles (2 kt each) so the first zT matmuls gate on
    # 128KB (Tile deps are tile-granular)
    bss_sb = [consts.tile([128, 2, M], BF, name=f"bss{i}") for i in range(4)]
    coef_sb = consts.tile([128, MT, 2, C], BF)
    owT_sb = consts.tile([128, CT * V], BF)
    maskT_sb = consts.tile([128, CT, L], FP32)
    crossb_sb = consts.tile([128, PCH], FP32)
    kscale_sb = consts.tile([128, 2], FP32)

    def dma_split(out_tile, in_ap, n, engines=(None,)):
        # split one big contiguous DMA into n pieces so they spread across
        # HWDGE queues (aggregate bandwidth), alternating the issuing engine
        # (each dma_start costs ~650ns serialized on its sequencer)
        if engines == (None,):
            engines = (nc.sync, nc.gpsimd)
        if len(out_tile.shape) == 3:
            g = out_tile.shape[1] // n
            b = out_tile.shape[2]
            for i in range(n):
                engines[i % len(engines)].dma_start(
                    out=out_tile[:, i * g:(i + 1) * g, :],
                    in_=in_ap[:, i * g * b:(i + 1) * g * b].rearrange(
                        "p (a b) -> p a b", b=b))
            return
        w = out_tile.shape[-1] // n
        for i in range(n):
            engines[i % len(engines)].dma_start(
                out=out_tile[:, i * w:(i + 1) * w],
                in_=in_ap[:, i * w:(i + 1) * w])

    # ---- persistent activations ----
    zT_sb = big.tile([128, MT, T_EXT], BF)     # [m, t] shared projection
    gT_sb = big.tile([128, MT, T_LOC], BF)     # [m, t] g = z @ G_a, main only
    z1T_sb = big.tile([128, MT, T_LOC], BF)    # crossb-scaled z~T
    gtil_sb = big.tile([128, NTT, M], BF)      # g~ = g * decay^t_local, [t, m]
    v_sb = big.tile([128, NTT, C], BF)         # v normal layout
    rT_sb = big.tile([128, CT, T_LOC], BF)     # retrieved^T

    # ---- startup DMAs: halo-x + first basis tiles first (gate the first
    # matmuls on the fewest bytes) ----
    start_engines = [nc.sync, nc.gpsimd, nc.scalar]
    xth = xpool.tile([128, KT * HALO], BF, tag="xth")
    pieces = [(xth[:, :KT * HALO // 2], xhh_d.ap()[:, :KT * HALO // 2]),
              (xth[:, KT * HALO // 2:], xhh_d.ap()[:, KT * HALO // 2:])]
    for i in range(4):
        pieces.append((bss_sb[i],
                       bss_d.ap()[:, i * 2 * M:(i + 1) * 2 * M].rearrange(
                           "p (a b) -> p a b", b=M)))
    for i in range(2):
        g = coef_d.ap().shape[1] // 2
        pieces.append((coef_sb[:, i, :, :],
                       coef_d.ap()[:, i * g:(i + 1) * g].rearrange(
                           "p (a b) -> p a b", b=C)))
    pieces.append((kscale_sb, kscale_d.ap()))
    for i, (dst, src) in enumerate(pieces):
        start_engines[i % 3].dma_start(out=dst, in_=src)

    # ---- phase 0: halo (128 pos): zT then fused k~|v then boundary state ----
    tt_h = T_LOC // 128  # halo t-tile index (16)
    for mt in range(MT):
        ps = psA.tile([128, HALO], FP32, tag="ps")
        for kt in range(KT):
            nc.tensor.matmul(
                ps, lhsT=bss_sb[kt // 2][:, kt % 2, mt * 128:(mt + 1) * 128],
                rhs=xth[:, kt * HALO:(kt + 1) * HALO],
                start=(kt == 0), stop=(kt == KT - 1))
        (nc.vector.tensor_copy if mt == 0 else nc.scalar.copy)(
            zT_sb[:, mt, T_LOC:T_EXT], ps)
    # fused g~|v for the halo block
    ps = psA.tile([128, 2 * C], FP32, tag="ps")
    for mt in range(MT):
        nc.tensor.matmul(
            ps, lhsT=zT_sb[:, mt, T_LOC:T_EXT],
            rhs=coef_sb[:, mt, :, :].rearrange("p a b -> p (a b)"),
            start=(mt == 0), stop=(mt == MT - 1))
    nc.vector.tensor_scalar_mul(
        gtil_sb[:, tt_h, :], ps[:, :M], kscale_sb[:, 0:1])
    nc.scalar.copy(v_sb[:, tt_h, :], ps[:, M:])
    # boundary state S~ = g~^T v over the halo block only
    S_cur = state.tile([128, MT, C], BF, tag="S")
    for st in range(MT):
        ps = psA.tile([128, C], FP32, tag="ps")
        nc.tensor.matmul(
            ps, lhsT=gtil_sb[:, tt_h, st * 128:(st + 1) * 128],
            rhs=v_sb[:, tt_h, :], start=True, stop=True)
        (nc.vector.tensor_copy if st == 0 else nc.scalar.copy)(
            S_cur[:, st, :], ps)

    # ---- main loop: projections (reverse t), retention, outproj ----
    def proj(pch, xt_a, xt_b):
        t0 = pch * PCH
        # zT: [m, t] via lhsT=basis tiles, rhs=xT
        # zT evacuated as two half copies on two engines so the dependent
        # gT/g~|v matmuls see bf16 zT ~350ns after the psum closes; z~T is
        # re-derived from SBUF off the critical path
        for mt in range(MT):
            ps = psA.tile([128, PCH], FP32, tag="ps")
            for kt in range(KT):
                xt = xt_a if kt < 4 else xt_b
                nc.tensor.matmul(
                    ps, lhsT=bss_sb[kt // 2][:, kt % 2, mt * 128:(mt + 1) * 128],
                    rhs=xt[:, (kt % 4) * PCH:(kt % 4 + 1) * PCH],
                    start=(kt == 0), stop=(kt == KT - 1))
            h = PCH // 2
            nc.scalar.copy(zT_sb[:, mt, t0:t0 + h], ps[:, :h])
            nc.vector.tensor_copy(zT_sb[:, mt, t0 + h:t0 + PCH], ps[:, h:])
            nc.vector.tensor_mul(
                z1T_sb[:, mt, t0:t0 + PCH], zT_sb[:, mt, t0:t0 + PCH],
                crossb_sb)
        # gT: [m, t] via lhsT=G_a blocks, rhs=zT; mt-major so the mt=1
        # accumulations land only after zT[mt=1]'s evacuation is done anyway
        gps = [psA.tile([128, PCH], FP32, tag="ps", name=f"gps{i}")
               for i in range(MT)]
        for mt in range(MT):
            for mmt in range(MT):
                nc.tensor.matmul(
                    gps[mmt], lhsT=coef_sb[:, mt, 0, mmt * 128:(mmt + 1) * 128],
                    rhs=zT_sb[:, mt, t0:t0 + PCH],
                    start=(mt == 0), stop=(mt == MT - 1))
        for mmt in range(MT):
            (nc.scalar.copy if mmt == 0 else nc.vector.tensor_copy)(
                gT_sb[:, mmt, t0:t0 + PCH], gps[mmt])
        # fused g~|v in normal [t, m] layout: lhsT=zT t-blocks, rhs=[G_a|vc],
        # mt-major in tb pairs
        for tp in range(2):
            tbs = (2 * tp, 2 * tp + 1)
            kps = [psA.tile([128, 2 * C], FP32, tag="ps", name=f"kps{tb}")
                   for tb in tbs]
            for mt in range(MT):
                for i, tb in enumerate(tbs):
                    nc.tensor.matmul(
                        kps[i],
                        lhsT=zT_sb[:, mt, t0 + tb * 128:t0 + (tb + 1) * 128],
                        rhs=coef_sb[:, mt, :, :].rearrange("p a b -> p (a b)"),
                        start=(mt == 0), stop=(mt == MT - 1))
            for i, tb in enumerate(tbs):
                tt = t0 // 128 + tb
                nc.vector.tensor_scalar_mul(
                    gtil_sb[:, tt, :], kps[i][:, :M],
                    kscale_sb[:, (tt % 2):(tt % 2) + 1])
                nc.scalar.copy(v_sb[:, tt, :], kps[i][:, M:])

    def retention_chunk(c):
        nonlocal S_cur
        c0 = c * L
        tt0 = c0 // 128  # first t-tile of this chunk (2 per chunk)
        # AT[j, i] = g_j . z_i ; masked -> atm (bf16).
        # jt=0 only needs i<128 (the j<128,i>=128 quarter is masked to zero).
        atm = []
        for jt in range(2):
            w = 128 if jt == 0 else L
            ps = psA.tile([128, w], FP32, tag="ps")
            for mt in range(MT):
                nc.tensor.matmul(
                    ps, lhsT=gT_sb[:, mt, c0 + jt * 128:c0 + (jt + 1) * 128],
                    rhs=zT_sb[:, mt, c0:c0 + w],
                    start=(mt == 0), stop=(mt == MT - 1))
            am = atmp.tile([128, w], BF, tag="atm")
            nc.vector.tensor_mul(am, ps, maskT_sb[:, jt, :w])
            atm.append(am)
        # cross part first: S~^T @ z~T needs no masks, so PE keeps running
        # while DVE applies them
        rps = []
        for ct in range(CT):
            ps = psB.tile([128, L], FP32, tag="ps")
            for st in range(MT):
                nc.tensor.matmul(
                    ps, lhsT=S_cur[:, st, ct * 128:(ct + 1) * 128],
                    rhs=z1T_sb[:, st, c0:c0 + L],
                    start=(st == 0), stop=False)
            rps.append(ps)
        # state S~ = g~^T v of THIS chunk only (decay^L * older state ~ 4e-6,
        # numerically negligible -> no recursion, chunks fully independent).
        # Emitted between the cross and intra parts as mask-independent filler.
        if c > 0:
            S_new = state.tile([128, MT, C], BF, tag="S")
            for st in range(MT):
                ps = psA.tile([128, C], FP32, tag="ps")
                for jt in range(2):
                    nc.tensor.matmul(
                        ps, lhsT=gtil_sb[:, tt0 + jt, st * 128:(st + 1) * 128],
                        rhs=v_sb[:, tt0 + jt, :],
                        start=(jt == 0), stop=(jt == 1))
                (nc.vector.tensor_copy if st == 0 else nc.scalar.copy)(
                    S_new[:, st, :], ps)
            S_cur = S_new
        # intra part: v^T @ atm accumulated into the same rT psum; jt=0 only
        # contributes to the first 128 columns
        for ct in range(CT):
            nc.tensor.matmul(
                rps[ct][:, :128], lhsT=v_sb[:, tt0, ct * 128:(ct + 1) * 128],
                rhs=atm[0], start=False, stop=False)
            nc.tensor.matmul(
                rps[ct], lhsT=v_sb[:, tt0 + 1, ct * 128:(ct + 1) * 128],
                rhs=atm[1], start=False, stop=True)
            (nc.vector.tensor_copy if ct == 0 else nc.scalar.copy)(
                rT_sb[:, ct, c0:c0 + L], rps[ct])

    def outproj(t0, width, final=False):
        # out[v, t0:t0+width]: owT^T @ rT; vt-pairs share one DMA (a 3D
        # [128, 2, w] staging tile maps onto 256 contiguous DRAM rows)
        for vt in range(KT):
            ps = psO.tile([128, PCH], FP32, tag="po")
            for ct in range(CT):
                nc.tensor.matmul(
                    ps[:, :width],
                    lhsT=owT_sb[:, ct * V + vt * 128:ct * V + (vt + 1) * 128],
                    rhs=rT_sb[:, ct, t0:t0 + width],
                    start=(ct == 0), stop=(ct == CT - 1))
            if vt % 2 == 0:
                ot = ostage.tile([128, 2, PCH], BF, tag="ot")
            # alternate evac engines so consecutive psO slots drain on two
            # engines concurrently
            (nc.vector.tensor_copy if vt % 2 == 0 else nc.scalar.copy)(
                ot[:, vt % 2, :width], ps[:, :width])
            if vt % 2 == 1:
                # the tail stays on sync HWDGE: a trailing gpsimd DMA makes
                # the kernel-end gpsimd DRAIN wait ~4us for SWDGE completion
                eng = nc.sync if (final or vt % 4 == 1) else nc.gpsimd
                eng.dma_start(
                    out=outT_d.ap()[(vt - 1) * 128:(vt + 1) * 128,
                                    t0:t0 + width].rearrange(
                        "(a p) b -> p a b", p=128),
                    in_=ot[:, :, :width])

    # x for pch=3 in two tiles so zT kt 0-3 gates on half the bytes
    xt3a = xpool.tile([128, 4 * PCH], BF, tag="xta")
    xt3b = xpool.tile([128, 4 * PCH], BF, tag="xtb")
    dma_split(xt3a, xh_d.ap()[3 * 128:4 * 128, :4 * PCH], 4,
              engines=(nc.sync, nc.gpsimd, nc.scalar))
    dma_split(xt3b, xh_d.ap()[3 * 128:4 * 128, 4 * PCH:], 4,
              engines=(nc.gpsimd, nc.sync))
    nc.gpsimd.dma_start(out=crossb_sb, in_=crossb_d.ap())
    nc.scalar.dma_start(out=maskT_sb,
                        in_=maskT_d.ap().rearrange("p (a b) -> p a b", b=L))

    # prefetch ALL remaining x chunks now (xpool holds 3 "xt" tiles), with
    # owT (first needed ~35us in) after xt1 in queue order
    xts = {3: (xt3a, xt3b)}
    for pch in (2, 1):
        xt = xpool.tile([128, KT * PCH], BF, tag="xt")
        dma_split(xt, xh_d.ap()[pch * 128:(pch + 1) * 128, :], 8)
        xts[pch] = (xt[:, :4 * PCH], xt[:, 4 * PCH:])
    dma_split(owT_sb, owT_d.ap(), 4)
    xt = xpool.tile([128, KT * PCH], BF, tag="xt")
    dma_split(xt, xh_d.ap()[0:128, :], 8)
    xts[0] = (xt[:, :4 * PCH], xt[:, 4 * PCH:])
    for pch in range(N_MAIN_PCH - 1, -1, -1):
        proj(pch, *xts[pch])
        if pch == N_MAIN_PCH - 1:
            retention_chunk(7)
            retention_chunk(6)
        else:
            # outproj for the previous pch's chunks, emitted one chunk after
            # its rT inputs complete so the rT evacuations have slack
            retention_chunk(2 * pch + 1)
            outproj((pch + 1) * PCH, PCH)
            retention_chunk(2 * pch)
    # tail at L granularity so the post-retention drain is short
    outproj(L, L)
    outproj(0, L, final=True)

    ctx.close()


# ---------------- host side ----------------

_NC_CACHE = None


def _get_nc():
    global _NC_CACHE
    if _NC_CACHE is None:
        _NC_CACHE = build_nc()
    return _NC_CACHE


def _tile128(a, inner):
    """[G*128, inner] -> [128, G*inner] with block g at columns [g*inner,...)."""
    g = a.shape[0] // 128
    return np.ascontiguousarray(
        a.reshape(g, 128, inner).transpose(1, 0, 2).reshape(128, g * inner))


def _prep_in_maps(inputs):
    x = np.asarray(inputs["x"], np.float32)
    basis = np.asarray(inputs["basis"], np.float32)
    decay = float(1.0 / (1.0 + np.exp(-np.float64(inputs["decay_logit"]))))
    out_scale = float(np.float32(inputs["out_scale"]))

    bss = _tile128(basis.astype(BF16), M)
    # coef[p, mt, 0, :] = G_a[mt*128+p, :] with G_a = kc^T @ qc;
    # coef[p, mt, 1, :] = vc^T[mt*128+p, :]
    qc = np.asarray(inputs["q_coeffs"], np.float32)
    kc = np.asarray(inputs["k_coeffs"], np.float32)
    vc = np.asarray(inputs["v_coeffs"], np.float32)
    Ga = (kc.T @ qc).astype(BF16)              # [M, M]
    coef = np.empty((128, MT, 2, C), dtype=BF16)
    for mt in range(MT):
        coef[:, mt, 0, :] = Ga[mt * 128:(mt + 1) * 128, :]
        coef[:, mt, 1, :] = vc.T[mt * 128:(mt + 1) * 128, :].astype(BF16)
    coef = np.ascontiguousarray(coef).reshape(128, MT * 2 * C)
    ow = basis @ np.asarray(inputs["o_coeffs"], np.float32).T
    owT = _tile128(np.ascontiguousarray((ow * out_scale).T).astype(BF16), V)

    i = np.arange(L)
    jj, ii = np.meshgrid(i, i, indexing="ij")
    maskT = np.where(jj > ii, decay ** np.maximum(jj - ii - 1, 0), 0.0).astype(np.float32)
    maskT = _tile128(maskT, L)
    cross = (decay ** (L - 1 - i)).astype(np.float32)
    crossb = np.ascontiguousarray(
        np.broadcast_to(np.tile(cross, PCH // L)[None, :], (128, PCH)), np.float32)
    ksc = decay ** np.arange(2 * 128, dtype=np.float64)
    kscale = np.stack([ksc[:128], ksc[128:]], axis=1).astype(np.float32)

    in_maps = []
    for core in range(N_CORES):
        b, h = divmod(core, 2)
        t0 = h * T_LOC
        te = min(t0 + T_EXT, T)
        xT = np.zeros((V, T_EXT), dtype=BF16)
        xT[:, :te - t0] = x[b, t0:te].T.astype(BF16)
        # pre-tile main: [pch][p][kt][t] contiguous; halo: [p][kt][t]
        xh = np.ascontiguousarray(
            xT[:, :T_LOC].reshape(KT, 128, N_MAIN_PCH, PCH).transpose(2, 1, 0, 3)
        ).reshape(N_MAIN_PCH * 128, KT * PCH)
        xhh = np.ascontiguousarray(
            xT[:, T_LOC:].reshape(KT, 128, HALO).transpose(1, 0, 2)
        ).reshape(128, KT * HALO)
        in_maps.append({
            "xh": xh, "xhh": xhh, "bss": bss, "coef": coef, "owT": owT,
            "maskT": maskT, "crossb": crossb, "kscale": kscale,
        })
    return in_maps


def _ensure_ntff_hook():
    """The agent image's antenv package lacks axon_hooks; shim it so
    run_bass_kernel_spmd(trace=True) can register the NTFF profile hook."""
    try:
        from antenv.axon_hooks import get_axon_ntff_profile_hook  # noqa: F401
        return
    except ImportError:
        pass
    import sys
    import types
    import antenv
    mod = types.ModuleType("antenv.axon_hooks")
    _state = {"hook": None}
    mod.set_axon_ntff_profile_hook = lambda h: _state.__setitem__("hook", h)
    mod.get_axon_ntff_profile_hook = lambda: _state["hook"]
    sys.modules["antenv.axon_hooks"] = mod
    antenv.axon_hooks = mod
    from trn_agent_boot.trn_boot import _ntff_profile_via_ctypes
    mod.set_axon_ntff_profile_hook(
        _ntff_profile_via_ctypes("/opt/axon/libaxon_pjrt.so"))


def run(inputs, trace=False):
    """Returns (out [B,T,V] float32, BassKernelResults)."""
    if trace:
        _ensure_ntff_hook()
    in_maps = _prep_in_maps(inputs)
    nc = _get_nc()
    res = run_bass_kernel_spmd(nc, in_maps, core_ids=list(range(N_CORES)),
                               trace=trace)
    out = np.zeros((B, T, V), np.float32)
    for core in range(N_CORES):
        b, h = divmod(core, 2)
        outT = np.asarray(res.results[core]["outT"]).astype(np.float32)
        out[b, h * T_LOC:(h + 1) * T_LOC] = outT.T
    return out, res


def kernel(**inputs):
    out, _ = run(inputs, trace=False)
    return out
